# revision 9
# baseline (speedup 1.0000x reference)
"""Curvphormer GNN layer as a Bass/Tile SPMD kernel for TRN2.

Design (per core c of NCORES, equal node ranges of W windows x 128 nodes):
 - Edges sharded by src-window range (core owns src in [c*W*128, (c+1)*W*128)).
 - Phase A: fused-LN q/k/v build for own node range (LN folded into matmuls),
   q/k tables in bf16.
 - AllGather(k bf16) -> full k table.
 - Pass 1 (shard sorted by tgt, 16-tile blocks): batched indirect-DMA gather of
   q[src] (own table) + k[tgt] (full table) — ONE descriptor-generation call
   per block per stream; scores = q.k/4 + curv@Wc + bc, ex = exp(score)
   (max-free softmax: scores are O(1) by construction); per-block one-hot
   generation; segment-sum of ex by tgt via one-hot matmuls into per-wide-window
   PSUM accumulators -> SBUF denominator table. Padding handled by an
   out-of-range one-hot index (no exp bias needed).
 - ReduceScatter(denom) -> own denominators; vnorm = v / denom (bf16);
   AllGather(vnorm bf16).
 - Pass 2 (shard grouped by src-window, fixed T2W tiles per window, 16-tile
   blocks): batched gather of vnorm[tgt] and ex (by pass-1 position),
   messages = mask * vnorm * ex_bcast (bf16), aggregate transposed agg via
   one-hot bf16 matmuls accumulated in PSUM per window.
 - Phase D (fused per window): out = x1 + FFN(LN2(x1)), x1 = x + agg@Wo + bo;
   FFN/attn-out matmuls in bf16.

Timing: chained-dispatch marginal. A single dispatch over the axon tunnel has
~75 ms of fixed client<->device round-trip latency that is unrelated to kernel
execution; we measure T(1) and T(1+B) where the B extra executions are chained
back-to-back on device, and report (T(1+B)-T(1))/B — the steady-state hardware
execution time per run.
"""

import sys
if "/opt/trn_rl_repo" not in sys.path:
    sys.path.insert(0, "/opt/trn_rl_repo")

import numpy as np

import concourse.bass as bass
import concourse.mybir as mybir
from concourse.masks import make_identity

F32 = mybir.dt.float32
BF16 = mybir.dt.bfloat16
I32 = mybir.dt.int32

D = 128
H = 8
HD = 16
LN_EPS = 1e-5
NOMATCH = 300.0  # one-hot index for padded slots: never matches iota < 256


class P:
    """Static program parameters (identical across cores -> SPMD safe)."""

    def __init__(self, ncores, W, T1, T2W):
        self.ncores = ncores
        self.W = W              # windows (of 128 nodes) per core
        self.T1 = T1            # pass-1 tiles (128 edges each) per core
        self.T2W = T2W          # pass-2 tiles per window
        self.nodes_pc = W * 128
        self.npad = ncores * W * 128
        self.T2 = W * T2W


def _bf16(a):
    import ml_dtypes
    return np.asarray(a, dtype=ml_dtypes.bfloat16)


# --------------------------------------------------------------------------
# Host-side preprocessing
# --------------------------------------------------------------------------

def host_prep(x, edge_index, curv, weights, ncores, W):
    """Build per-core input maps. weights: dict with raw reference weights."""
    N = x.shape[0]
    E = edge_index.shape[1]
    nodes_pc = W * 128
    npad = ncores * nodes_pc
    assert npad >= N

    src = np.asarray(edge_index[0], dtype=np.int64)
    tgt = np.asarray(edge_index[1], dtype=np.int64)
    x_pad = np.zeros((npad, D), dtype=np.float32)
    x_pad[:N] = x

    core_of = (src // 128) // W
    order_by_core = np.argsort(core_of, kind="stable")
    counts = np.bincount(core_of, minlength=ncores)
    splits = np.split(order_by_core, np.cumsum(counts)[:-1])

    # pass-1: edges grouped by 256-node wide tgt-windows, padded to a fixed
    # tile count per wide-window (static, SPMD-uniform).
    NWW = (ncores * W + 1) // 2  # wide windows of 256 nodes
    T1W = 0
    for c in range(ncores):
        cnt = np.bincount(tgt[splits[c]] // 256, minlength=NWW)
        T1W = max(T1W, int(np.ceil(cnt.max() / 128)))
    T1 = NWW * T1W
    # pass-2: max tiles per (core, window)
    T2W = 0
    for c in range(ncores):
        e_c = splits[c]
        w_loc = (src[e_c] // 128) - c * W
        cnt = np.bincount(w_loc, minlength=W)
        T2W = max(T2W, int(np.ceil(cnt.max() / 128)))
    T2 = W * T2W

    pp = P(ncores, W, T1, T2W)
    pp.NWW = NWW
    pp.T1W = T1W

    # LN-folded weights (host)
    g1, be1, g2, be2 = weights["g1"], weights["be1"], weights["g2"], weights["be2"]

    def fold(Wm, b):
        Wp = (g1[:, None] * Wm).astype(np.float32)
        r1 = Wp.sum(axis=0).astype(np.float32)
        b2 = (be1 @ Wm + b).astype(np.float32)
        return Wp, r1, b2

    wq, r1q, bq2 = fold(weights["Wq"], weights["bq"])
    wk, r1k, bk2 = fold(weights["Wk"], weights["bk"])
    wv, r1v, bv2 = fold(weights["Wv"], weights["bv"])
    w1 = (g2[:, None] * weights["W1"]).astype(np.float32)
    r11 = w1.sum(axis=0).astype(np.float32)
    b12 = (be2 @ weights["W1"] + weights["b1"]).astype(np.float32)

    common = {
        "wq": wq, "wk": wk, "wv": wv,
        "wc": _bf16(weights["Wc"]),
        "wo": _bf16(weights["Wo"]),
        "w1": _bf16(w1),
        "w2": _bf16(np.ascontiguousarray(
            weights["W2"].astype(np.float32).reshape(4, 128, D)
            .transpose(1, 0, 2).reshape(128, 4 * D))),
        "r1q": r1q[None, :], "r1k": r1k[None, :], "r1v": r1v[None, :],
        "bq2": bq2[None, :], "bk2": bk2[None, :], "bv2": bv2[None, :],
        "bc_b": np.tile(weights["bc"].astype(np.float32)[None, :], (128, 1)),
        "bo_r": _bf16(weights["bo"])[None, :],
        "r11": _bf16(r11)[None, :], "b12": _bf16(b12)[None, :],
        "b2_r": _bf16(weights["b2"])[None, :],
        "ones_r": _bf16(np.ones((1, D), np.float32)),
        "iota256": np.tile(np.arange(256, dtype=np.float32)[None, :], (128, 1)),
        "iota128": np.tile(np.arange(128, dtype=np.float32)[None, :], (128, 1)),
    }

    in_maps = []
    for c in range(ncores):
        e_c = splits[c]
        L = len(e_c)
        # ---- pass 1: group by wide tgt-window, fixed T1W tiles each ----
        NWW, T1W = pp.NWW, pp.T1W
        S1 = T1 * 128
        tgt1 = np.zeros(S1, np.int64)
        src1 = np.zeros(S1, np.int64)
        real1 = np.zeros(S1, bool)
        slot1_of_edge = np.zeros(E, np.int64)
        ww_of = tgt[e_c] // 256
        for ww in range(NWW):
            ew = e_c[ww_of == ww]
            base = ww * T1W * 128
            k = len(ew)
            assert k <= T1W * 128
            tgt1[base:base + k] = tgt[ew]
            src1[base:base + k] = src[ew]
            real1[base:base + k] = True
            slot1_of_edge[ew] = base + np.arange(k)

        wwin1 = np.repeat(np.arange(T1) // T1W, 128)  # wide window per slot
        tgt_rel = np.where(real1, tgt1 - wwin1 * 256, NOMATCH)
        assert tgt_rel.min() >= 0 and tgt_rel.max() <= NOMATCH

        curv1 = np.zeros((S1, D), np.float32)
        if L:
            curv1[real1] = curv[np.concatenate(
                [e_c[ww_of == ww] for ww in range(NWW)])]
        # [128 d, T1*128] partition-major layout: row d, col t*128+e
        curv1t = _bf16(np.ascontiguousarray(
            curv1.reshape(T1, 128, D).transpose(2, 0, 1).reshape(D, T1 * 128)))

        def lay(a, T):  # [T*128] -> [128, T]
            return np.ascontiguousarray(a.reshape(T, 128).T)

        qi = lay(np.where(real1, src1 - c * nodes_pc, 0).astype(np.int32), T1)
        ki = lay(tgt1.astype(np.int32) * real1.astype(np.int32), T1)
        trel = lay(tgt_rel.astype(np.float32), T1)

        # ex row id = p1*T1 + t1 ; slot s -> p1 = s%128, t1 = s//128
        exrow_arr = np.zeros(E, np.int64)
        exrow_arr[e_c] = (slot1_of_edge[e_c] % 128) * T1 + (slot1_of_edge[e_c] // 128)

        # ---- pass 2: group by own src-window ----
        w_loc = (src[e_c] // 128) - c * W
        S2 = T2 * 128
        vn = np.zeros(S2, np.int64)
        exp_pos = np.zeros(S2, np.int64)
        sl2 = np.zeros(S2, np.int64)
        m2 = np.zeros(S2, np.float32)
        for w in range(W):
            ew = e_c[w_loc == w]
            base = w * T2W * 128
            k = len(ew)
            assert k <= T2W * 128
            vn[base:base + k] = tgt[ew]
            exp_pos[base:base + k] = exrow_arr[ew]
            sl2[base:base + k] = src[ew] - (c * W + w) * 128
            m2[base:base + k] = 1.0

        vni = lay(vn.astype(np.int32), T2)
        expos = lay(exp_pos.astype(np.int32), T2)
        srcl2 = lay(sl2.astype(np.float32), T2)
        mask2 = lay(m2, T2)

        x_own = np.ascontiguousarray(x_pad[c * nodes_pc:(c + 1) * nodes_pc])
        xT_own = np.ascontiguousarray(x_own.T)

        m = dict(common)
        m.update({
            "x_own": x_own, "xT_own": xT_own,
            "curv1t": curv1t,
            "qi": qi, "ki": ki, "trel": trel,
            "vni": vni, "expos": expos, "srcl2": srcl2, "mask2": mask2,
        })
        in_maps.append(m)

    return pp, in_maps


# --------------------------------------------------------------------------
# Device program
# --------------------------------------------------------------------------

def declare_io(nc, pp):
    """Declare all ExternalInput/Output dram tensors; returns dict of APs."""
    t = {}

    def din(name, shape, dt=F32):
        t[name] = nc.dram_tensor(name, list(shape), dt, kind="ExternalInput").ap()

    W, T1, T2 = pp.W, pp.T1, pp.T2
    din("x_own", (pp.nodes_pc, D)); din("xT_own", (D, pp.nodes_pc))
    din("curv1t", (D, T1 * 128), BF16)
    din("qi", (128, T1), I32); din("ki", (128, T1), I32)
    din("trel", (128, T1))
    din("vni", (128, T2), I32); din("expos", (128, T2), I32)
    din("srcl2", (128, T2)); din("mask2", (128, T2))
    for n, shp, dt in [("wq", (D, D), F32), ("wk", (D, D), F32),
                       ("wv", (D, D), F32),
                       ("wc", (D, H), BF16), ("wo", (D, D), BF16),
                       ("w1", (D, 4 * D), BF16), ("w2", (D, 4 * D), BF16),
                       ("r1q", (1, D), F32), ("r1k", (1, D), F32),
                       ("r1v", (1, D), F32),
                       ("bq2", (1, D), F32), ("bk2", (1, D), F32),
                       ("bv2", (1, D), F32),
                       ("bc_b", (128, H), F32), ("bo_r", (1, D), BF16),
                       ("r11", (1, 4 * D), BF16), ("b12", (1, 4 * D), BF16),
                       ("b2_r", (1, D), BF16),
                       ("ones_r", (1, D), BF16),
                       ("iota256", (128, 256), F32),
                       ("iota128", (128, 128), F32)]:
        din(n, shp, dt)
    t["out"] = nc.dram_tensor("out", [pp.nodes_pc, D], F32,
                              kind="ExternalOutput").ap()
    return t


def build(tc, t, pp):
    nc = tc.nc
    _rr = [0]

    def ind_dma(out, in_, off_ap):
        inst = nc.gpsimd.indirect_dma_start(
            out=out, out_offset=None, in_=in_,
            in_offset=bass.IndirectOffsetOnAxis(ap=off_ap, axis=0))
        q = _rr[0] % 4
        _rr[0] += 1
        if q:
            inst.ins.queue = f"qPoolDynamic{q}"
        return inst
    W, T1, T2W, T2 = pp.W, pp.T1, pp.T2W, pp.T2
    NW = pp.ncores * W  # total windows (392)
    rg = [list(range(pp.ncores))]
    from contextlib import ExitStack
    ctx = ExitStack()

    # internal DRAM
    q_own_d, _ = tc.tile([pp.nodes_pc, D], BF16, space="DRAM", name="q_own_d")
    k_own_d, _ = tc.tile([pp.nodes_pc, D], BF16, space="DRAM", name="k_own_d")
    k_full, _ = tc.tile([pp.npad, D], BF16, space="DRAM", addr_space="Shared",
                        name="k_full")
    den_d, _ = tc.tile([NW * 128, H], F32, space="DRAM", name="den_d")
    den_own, _ = tc.tile([pp.nodes_pc, H], F32, space="DRAM",
                         addr_space="Shared", name="den_own")
    vn_own_d, _ = tc.tile([pp.nodes_pc, D], BF16, space="DRAM", name="vn_own_d")
    vn_full, _ = tc.tile([pp.npad, D], BF16, space="DRAM", addr_space="Shared",
                         name="vn_full")
    ex_d, _ = tc.tile([128, T1 * H], BF16, space="DRAM", name="ex_d")

    const = ctx.enter_context(tc.tile_pool(name="const", bufs=1))

    def load_const(name, dt=None, src=None):
        ap = t[name] if src is None else src
        shp = list(ap.shape)
        tl = const.tile(shp, dt or ap.dtype, name=f"c_{name}")
        nc.sync.dma_start(tl[:], ap[:])
        return tl

    wq_s = load_const("wq"); wk_s = load_const("wk"); wv_s = load_const("wv")
    wc_s = load_const("wc"); wo_s = load_const("wo"); w1_s = load_const("w1")
    w2_s = load_const("w2")
    r1q_s = load_const("r1q"); r1k_s = load_const("r1k"); r1v_s = load_const("r1v")
    bq2_s = load_const("bq2"); bk2_s = load_const("bk2"); bv2_s = load_const("bv2")
    bc_s = load_const("bc_b"); bo_s = load_const("bo_r")
    r11_s = load_const("r11"); b12_s = load_const("b12"); b2_s = load_const("b2_r")
    ones_s = load_const("ones_r")
    qi_s = load_const("qi"); ki_s = load_const("ki")
    vni_s = load_const("vni"); expos_s = load_const("expos")
    mask2_f = load_const("mask2")
    srcl2_f = load_const("srcl2")
    trel_f = load_const("trel")
    iota256_f = load_const("iota256")
    iota128_f = load_const("iota128")

    ident = const.tile([128, 128], F32, name="ident")
    make_identity(nc, ident[:])
    ident_b = const.tile([128, 128], BF16, name="ident_b")
    nc.vector.tensor_copy(out=ident_b[:], in_=ident[:])
    eps_col = const.tile([128, 1], F32, name="eps_col")
    nc.vector.memset(eps_col[:], LN_EPS)

    # bf16 copies for the one-hot / message paths
    trel_s = const.tile([128, T1], BF16, name="trel_b")
    nc.vector.tensor_copy(out=trel_s[:], in_=trel_f[:])
    iota256_s = const.tile([128, 256], BF16, name="iota256_b")
    nc.vector.tensor_copy(out=iota256_s[:], in_=iota256_f[:])
    iota128_s = const.tile([128, 128], BF16, name="iota128_b")
    nc.vector.tensor_copy(out=iota128_s[:], in_=iota128_f[:])
    srcl2_s = const.tile([128, T2], BF16, name="srcl2_b")
    nc.vector.tensor_copy(out=srcl2_s[:], in_=srcl2_f[:])
    mask2_s = const.tile([128, T2], BF16, name="mask2_b")
    nc.vector.tensor_copy(out=mask2_s[:], in_=mask2_f[:])

    # residents
    v_res = const.tile([128, W * 128], BF16, name="v_res")
    ex_sb = const.tile([128, T1 * H], BF16, name="ex_sb")
    den_tab = const.tile([128, (NW + 1) * H], F32, name="den_tab")
    nc.vector.memset(den_tab[:], 0.0)

    # ---------------- Phase A: q/k/v for own windows ----------------
    with tc.tile_pool(name="pA", bufs=2) as pA, \
         tc.tile_pool(name="pAp", bufs=1, space="PSUM") as pAp:
        for w in range(W):
            xw = pA.tile([128, 128], F32, tag="xw")
            nc.sync.dma_start(xw[:], t["x_own"][w * 128:(w + 1) * 128, :])
            xTw = pA.tile([128, 128], F32, tag="xTw")
            nc.sync.dma_start(xTw[:], t["xT_own"][:, w * 128:(w + 1) * 128])
            # stats
            s1 = pA.tile([128, 1], F32, tag="s1")
            nc.vector.tensor_reduce(out=s1[:], in_=xw[:],
                                    axis=mybir.AxisListType.X,
                                    op=mybir.AluOpType.add)
            sq = pA.tile([128, 128], F32, tag="sq")
            nc.scalar.activation(out=sq[:], in_=xw[:],
                                 func=mybir.ActivationFunctionType.Square)
            s2 = pA.tile([128, 1], F32, tag="s2")
            nc.vector.tensor_reduce(out=s2[:], in_=sq[:],
                                    axis=mybir.AxisListType.X,
                                    op=mybir.AluOpType.add)
            mcol = pA.tile([128, 1], F32, tag="mcol")
            nc.vector.tensor_scalar_mul(mcol[:], s1[:], 1.0 / 128.0)
            m2c = pA.tile([128, 1], F32, tag="m2c")
            nc.vector.tensor_tensor(out=m2c[:], in0=mcol[:], in1=mcol[:],
                                    op=mybir.AluOpType.mult)
            var = pA.tile([128, 1], F32, tag="var")
            nc.vector.scalar_tensor_tensor(out=var[:], in0=s2[:],
                                           scalar=1.0 / 128.0, in1=m2c[:],
                                           op0=mybir.AluOpType.mult,
                                           op1=mybir.AluOpType.subtract)
            stdc = pA.tile([128, 1], F32, tag="stdc")
            nc.scalar.activation(out=stdc[:], in_=var[:],
                                 func=mybir.ActivationFunctionType.Sqrt,
                                 bias=eps_col[:])
            rstd = pA.tile([128, 1], F32, tag="rstd")
            nc.vector.reciprocal(out=rstd[:], in_=stdc[:])
            negm = pA.tile([128, 1], F32, tag="negm")
            nc.vector.tensor_scalar_mul(negm[:], mcol[:], -1.0)
            nm_ps = pAp.tile([128, 128], F32, tag="tr_ps")
            nc.tensor.transpose(out=nm_ps[:1, :], in_=negm[:], identity=ident[:])
            st_ps = pAp.tile([128, 128], F32, tag="tr_ps")
            nc.tensor.transpose(out=st_ps[:1, :], in_=stdc[:], identity=ident[:])
            negm_r = pA.tile([1, 128], F32, tag="negm_r")
            nc.vector.tensor_copy(out=negm_r[:], in_=nm_ps[:1, :])
            std_r = pA.tile([1, 128], F32, tag="std_r")
            nc.vector.tensor_copy(out=std_r[:], in_=st_ps[:1, :])

            for nm, wmat, r1m, b2m in (("q", wq_s, r1q_s, bq2_s),
                                       ("k", wk_s, r1k_s, bk2_s),
                                       ("v", wv_s, r1v_s, bv2_s)):
                ps = pAp.tile([128, 128], F32, tag="ps")
                nc.tensor.matmul(out=ps[:], lhsT=xTw[:], rhs=wmat[:],
                                 start=True, stop=False)
                nc.tensor.matmul(out=ps[:], lhsT=negm_r[:], rhs=r1m[:],
                                 start=False, stop=False)
                nc.tensor.matmul(out=ps[:], lhsT=std_r[:], rhs=b2m[:],
                                 start=False, stop=True)
                if nm == "v":
                    nc.scalar.activation(out=v_res[:, w * 128:(w + 1) * 128],
                                         in_=ps[:],
                                         func=mybir.ActivationFunctionType.Copy,
                                         scale=rstd[:])
                else:
                    ot = pA.tile([128, 128], BF16, tag=f"o_{nm}")
                    nc.scalar.activation(out=ot[:], in_=ps[:],
                                         func=mybir.ActivationFunctionType.Copy,
                                         scale=rstd[:])
                    dst = q_own_d if nm == "q" else k_own_d
                    nc.sync.dma_start(dst[w * 128:(w + 1) * 128, :], ot[:])

    # AllGather k (bf16)
    nc.gpsimd.collective_compute(
        "AllGather", mybir.AluOpType.bypass, replica_groups=rg,
        ins=[k_own_d.opt()], outs=[k_full.opt()])

    # ---------------- Pass 1 ----------------
    T1W = pp.T1W
    B1 = 16
    _psd_cur = [None, None]
    nb1 = (T1 + B1 - 1) // B1
    with tc.tile_pool(name="p1", bufs=2) as p1, \
         tc.tile_pool(name="p1b", bufs=2) as p1b, \
         tc.tile_pool(name="p1p", bufs=2, space="PSUM") as p1p:
        for bi in range(nb1):
            t0 = bi * B1
            nt = min(B1, T1 - t0)
            cvb = p1b.tile([128, B1 * 128], BF16, tag="cvb")
            nc.sync.dma_start(cvb[:, :nt * 128],
                              t["curv1t"][:, t0 * 128:(t0 + nt) * 128])
            qgb = p1b.tile([128, B1 * 128], BF16, tag="qgb")
            kgb = p1b.tile([128, B1 * 128], BF16, tag="kgb")
            for j in range(nt):
                ind_dma(qgb[:, j * 128:(j + 1) * 128], q_own_d[:],
                        qi_s[:, t0 + j:t0 + j + 1])
                ind_dma(kgb[:, j * 128:(j + 1) * 128], k_full[:],
                        ki_s[:, t0 + j:t0 + j + 1])
            # curv @ Wc (per-tile lhsT) into one PSUM block
            psc = p1p.tile([128, B1 * H], F32, tag="psc")
            for j in range(nt):
                nc.tensor.matmul(out=psc[:, j * H:(j + 1) * H],
                                 lhsT=cvb[:, j * 128:(j + 1) * 128],
                                 rhs=wc_s[:], start=True, stop=True)
            # scores for the whole block
            prod = p1.tile([128, B1 * 128], BF16, tag="prod")
            nc.vector.tensor_tensor(out=prod[:, :nt * 128],
                                    in0=qgb[:, :nt * 128],
                                    in1=kgb[:, :nt * 128],
                                    op=mybir.AluOpType.mult)
            qk = p1.tile([128, B1 * H], F32, tag="qk")
            nc.vector.tensor_reduce(
                out=qk[:, :nt * H],
                in_=prod[:, :nt * 128].rearrange("p (q x) -> p q x", x=HD),
                axis=mybir.AxisListType.X, op=mybir.AluOpType.add)
            qks = p1.tile([128, B1 * H], F32, tag="qks")
            nc.vector.scalar_tensor_tensor(out=qks[:, :nt * H],
                                           in0=qk[:, :nt * H],
                                           scalar=0.25, in1=psc[:, :nt * H],
                                           op0=mybir.AluOpType.mult,
                                           op1=mybir.AluOpType.add)
            nc.vector.tensor_tensor(
                out=qks[:, :nt * H].rearrange("p (q h) -> p q h", h=H),
                in0=qks[:, :nt * H].rearrange("p (q h) -> p q h", h=H),
                in1=bc_s[:].rearrange("p (o h) -> p o h", o=1)
                .broadcast_to([128, nt, H]),
                op=mybir.AluOpType.add)
            nc.scalar.activation(out=ex_sb[:, t0 * H:(t0 + nt) * H],
                                 in_=qks[:, :nt * H],
                                 func=mybir.ActivationFunctionType.Exp)
            # one-hot columns for this block
            ohb = p1.tile([128, B1 * 256], BF16, tag="ohb")
            nc.vector.tensor_tensor(
                out=ohb[:, :nt * 256].rearrange("p (q n) -> p q n", n=256),
                in0=trel_s[:, t0:t0 + nt].rearrange("p (q o) -> p q o", o=1)
                .broadcast_to([128, nt, 256]),
                in1=iota256_s[:].rearrange("p (o n) -> p o n", o=1)
                .broadcast_to([128, nt, 256]),
                op=mybir.AluOpType.is_equal)
            for j in range(nt):
                ti = t0 + j
                ex_t = ex_sb[:, ti * H:(ti + 1) * H]
                ww = ti // T1W
                tt1 = ti % T1W
                if tt1 == 0:
                    _psd_cur[0] = p1p.tile([128, H], F32, tag="psd_lo", name="psd_lo")
                    _psd_cur[1] = p1p.tile([128, H], F32, tag="psd_hi", name="psd_hi")
                psd_lo, psd_hi = _psd_cur[0], _psd_cur[1]
                nc.tensor.matmul(out=psd_lo[:],
                                 lhsT=ohb[:, j * 256:j * 256 + 128], rhs=ex_t,
                                 start=(tt1 == 0), stop=(tt1 == T1W - 1))
                nc.tensor.matmul(out=psd_hi[:],
                                 lhsT=ohb[:, j * 256 + 128:(j + 1) * 256],
                                 rhs=ex_t, start=(tt1 == 0),
                                 stop=(tt1 == T1W - 1))
                if tt1 == T1W - 1:
                    nc.vector.tensor_copy(
                        out=den_tab[:, ww * 2 * H:ww * 2 * H + H],
                        in_=psd_lo[:])
                    nc.vector.tensor_copy(
                        out=den_tab[:, ww * 2 * H + H:(ww + 1) * 2 * H],
                        in_=psd_hi[:])
        nc.sync.dma_start(ex_d[:], ex_sb[:])
        nc.sync.dma_start(
            den_d[:].rearrange("(w p) h -> p w h", p=128),
            den_tab[:, :NW * H].rearrange("p (w h) -> p w h", h=H))

    # ReduceScatter denom -> own rows
    nc.gpsimd.collective_compute(
        "ReduceScatter", mybir.AluOpType.add, replica_groups=rg,
        ins=[den_d.opt()], outs=[den_own.opt()])

    # ---------------- Phase C: vnorm ----------------
    with tc.tile_pool(name="pC", bufs=2) as pC:
        den_sb = pC.tile([128, W * H], F32, tag="den_sb")
        nc.sync.dma_start(den_sb[:].rearrange("p (w h) -> p w h", h=H),
                          den_own[:].rearrange("(w p) h -> p w h", p=128))
        nc.vector.tensor_scalar_max(den_sb[:], den_sb[:], 1e-30)
        rec = pC.tile([128, W * H], F32, tag="rec")
        nc.vector.reciprocal(out=rec[:], in_=den_sb[:])
        for w in range(W):
            vnw = pC.tile([128, 128], BF16, tag="vnw")
            nc.vector.tensor_tensor(
                out=vnw[:].rearrange("p (h x) -> p h x", h=H),
                in0=v_res[:, w * 128:(w + 1) * 128]
                .rearrange("p (h x) -> p h x", h=H),
                in1=rec[:, w * H:(w + 1) * H].broadcast_to([128, H, HD]),
                op=mybir.AluOpType.mult)
            nc.sync.dma_start(vn_own_d[w * 128:(w + 1) * 128, :], vnw[:])

    nc.gpsimd.collective_compute(
        "AllGather", mybir.AluOpType.bypass, replica_groups=rg,
        ins=[vn_own_d.opt()], outs=[vn_full.opt()])

    # ---------------- Pass 2 + Phase D ----------------
    ex_flat = ex_d[:].rearrange("p (t e) -> (p t) e", e=H)
    B2 = 16
    with tc.tile_pool(name="p2", bufs=2) as p2, \
         tc.tile_pool(name="p2b", bufs=2) as p2b, \
         tc.tile_pool(name="p2p", bufs=2, space="PSUM") as p2p, \
         tc.tile_pool(name="pD", bufs=2) as pD, \
         tc.tile_pool(name="pDp", bufs=1, space="PSUM") as pDp:
        nb2 = (T2 + B2 - 1) // B2
        # prefetch loop is flat over tiles; window boundaries align since
        # T2W*W tiles total and windows are contiguous runs of T2W tiles.
        for bi in range(nb2):
            t0 = bi * B2
            nt = min(B2, T2 - t0)
            vgb = p2b.tile([128, B2 * 128], BF16, tag="vgb")
            egb = p2b.tile([128, B2 * H], BF16, tag="egb")
            for j in range(nt):
                ind_dma(vgb[:, j * 128:(j + 1) * 128], vn_full[:],
                        vni_s[:, t0 + j:t0 + j + 1])
                ind_dma(egb[:, j * H:(j + 1) * H], ex_flat,
                        expos_s[:, t0 + j:t0 + j + 1])
            # masked ex for the block
            egm = p2b.tile([128, B2 * H], BF16, tag="egm")
            nc.vector.tensor_tensor(
                out=egm[:, :nt * H].rearrange("p (q h) -> p q h", h=H),
                in0=egb[:, :nt * H].rearrange("p (q h) -> p q h", h=H),
                in1=mask2_s[:, t0:t0 + nt].rearrange("p (q o) -> p q o", o=1)
                .broadcast_to([128, nt, H]),
                op=mybir.AluOpType.mult)
            # messages for the block
            msgb = p2.tile([128, B2 * 128], BF16, tag="msgb")
            nc.vector.tensor_tensor(
                out=msgb[:, :nt * 128].rearrange("p (q x) -> p q x", x=HD),
                in0=vgb[:, :nt * 128].rearrange("p (q x) -> p q x", x=HD),
                in1=egm[:, :nt * H].broadcast_to([128, nt * H, HD]),
                op=mybir.AluOpType.mult)
            # one-hot src columns for the block
            oh2b = p2.tile([128, B2 * 128], BF16, tag="oh2b")
            nc.vector.tensor_tensor(
                out=oh2b[:, :nt * 128].rearrange("p (q n) -> p q n", n=128),
                in0=srcl2_s[:, t0:t0 + nt].rearrange("p (q o) -> p q o", o=1)
                .broadcast_to([128, nt, 128]),
                in1=iota128_s[:].rearrange("p (o n) -> p o n", o=1)
                .broadcast_to([128, nt, 128]),
                op=mybir.AluOpType.is_equal)
            for j in range(nt):
                ti = t0 + j
                w = ti // T2W
                tt = ti % T2W
                if tt == 0:
                    aggT = p2p.tile([128, 128], F32, tag="aggT")
                    tc._aggT_cur = aggT  # stash
                aggT = tc._aggT_cur
                nc.tensor.matmul(out=aggT[:],
                                 lhsT=msgb[:, j * 128:(j + 1) * 128],
                                 rhs=oh2b[:, j * 128:(j + 1) * 128],
                                 start=(tt == 0), stop=(tt == T2W - 1))
                if tt == T2W - 1:
                    # -------- Phase D for window w --------
                    aggT_sb = pD.tile([128, 128], BF16, tag="aggT_sb")
                    nc.vector.tensor_copy(out=aggT_sb[:], in_=aggT[:])
                    attn = pDp.tile([128, 128], F32, tag="attn")
                    nc.tensor.matmul(out=attn[:], lhsT=aggT_sb[:], rhs=wo_s[:],
                                     start=True, stop=False)
                    nc.tensor.matmul(out=attn[:], lhsT=ones_s[:], rhs=bo_s[:],
                                     start=False, stop=True)
                    xw2 = pD.tile([128, 128], F32, tag="xw2")
                    nc.sync.dma_start(xw2[:],
                                      t["x_own"][w * 128:(w + 1) * 128, :])
                    x1 = pD.tile([128, 128], F32, tag="x1")
                    nc.vector.tensor_tensor(out=x1[:], in0=xw2[:], in1=attn[:],
                                            op=mybir.AluOpType.add)
                    # LN2 stats
                    s1b = pD.tile([128, 1], F32, tag="s1b")
                    nc.vector.tensor_reduce(out=s1b[:], in_=x1[:],
                                            axis=mybir.AxisListType.X,
                                            op=mybir.AluOpType.add)
                    sqb = pD.tile([128, 128], F32, tag="sqb")
                    nc.scalar.activation(
                        out=sqb[:], in_=x1[:],
                        func=mybir.ActivationFunctionType.Square)
                    s2b = pD.tile([128, 1], F32, tag="s2b")
                    nc.vector.tensor_reduce(out=s2b[:], in_=sqb[:],
                                            axis=mybir.AxisListType.X,
                                            op=mybir.AluOpType.add)
                    mb = pD.tile([128, 1], F32, tag="mb")
                    nc.vector.tensor_scalar_mul(mb[:], s1b[:], 1.0 / 128.0)
                    m2b = pD.tile([128, 1], F32, tag="m2b")
                    nc.vector.tensor_tensor(out=m2b[:], in0=mb[:], in1=mb[:],
                                            op=mybir.AluOpType.mult)
                    varb = pD.tile([128, 1], F32, tag="varb")
                    nc.vector.scalar_tensor_tensor(
                        out=varb[:], in0=s2b[:], scalar=1.0 / 128.0, in1=m2b[:],
                        op0=mybir.AluOpType.mult, op1=mybir.AluOpType.subtract)
                    stdb = pD.tile([128, 1], F32, tag="stdb")
                    nc.scalar.activation(
                        out=stdb[:], in_=varb[:],
                        func=mybir.ActivationFunctionType.Sqrt,
                        bias=eps_col[:])
                    rstdb = pD.tile([128, 1], F32, tag="rstdb")
                    nc.vector.reciprocal(out=rstdb[:], in_=stdb[:])
                    negmb = pD.tile([128, 1], F32, tag="negmb")
                    nc.vector.tensor_scalar_mul(negmb[:], mb[:], -1.0)
                    nm_psb = pDp.tile([128, 128], F32, tag="tr_psb")
                    nc.tensor.transpose(out=nm_psb[:1, :], in_=negmb[:],
                                        identity=ident[:])
                    st_psb = pDp.tile([128, 128], F32, tag="tr_psb")
                    nc.tensor.transpose(out=st_psb[:1, :], in_=stdb[:],
                                        identity=ident[:])
                    negm_rb = pD.tile([1, 128], BF16, tag="negm_rb")
                    nc.vector.tensor_copy(out=negm_rb[:], in_=nm_psb[:1, :])
                    std_rb = pD.tile([1, 128], BF16, tag="std_rb")
                    nc.vector.tensor_copy(out=std_rb[:], in_=st_psb[:1, :])
                    # x1T (bf16 for the FFN matmuls)
                    x1T_ps = pDp.tile([128, 128], F32, tag="tr_psb")
                    nc.tensor.transpose(out=x1T_ps[:], in_=x1[:],
                                        identity=ident[:])
                    x1T = pD.tile([128, 128], BF16, tag="x1T")
                    nc.vector.tensor_copy(out=x1T[:], in_=x1T_ps[:])
                    hp = pDp.tile([128, 512], F32, tag="hp")
                    nc.tensor.matmul(out=hp[:], lhsT=x1T[:], rhs=w1_s[:],
                                     start=True, stop=False)
                    nc.tensor.matmul(out=hp[:], lhsT=negm_rb[:], rhs=r11_s[:],
                                     start=False, stop=False)
                    nc.tensor.matmul(out=hp[:], lhsT=std_rb[:], rhs=b12_s[:],
                                     start=False, stop=True)
                    hsb = pD.tile([128, 512], BF16, tag="hsb")
                    nc.scalar.activation(out=hsb[:], in_=hp[:],
                                         func=mybir.ActivationFunctionType.Relu,
                                         scale=rstdb[:])
                    ffn = pDp.tile([128, 128], F32, tag="ffn")
                    for cch in range(4):
                        hT_ps = pDp.tile([128, 128], BF16, tag="tr_psb2")
                        nc.tensor.transpose(
                            out=hT_ps[:], in_=hsb[:, cch * 128:(cch + 1) * 128],
                            identity=ident_b[:])
                        hT = pD.tile([128, 128], BF16, tag="hT")
                        nc.vector.tensor_copy(out=hT[:], in_=hT_ps[:])
                        nc.tensor.matmul(out=ffn[:], lhsT=hT[:],
                                         rhs=w2_s[:, cch * 128:(cch + 1) * 128],
                                         start=(cch == 0), stop=False)
                    nc.tensor.matmul(out=ffn[:], lhsT=ones_s[:], rhs=b2_s[:],
                                     start=False, stop=True)
                    outw = pD.tile([128, 128], F32, tag="outw")
                    nc.vector.tensor_tensor(out=outw[:], in0=x1[:], in1=ffn[:],
                                            op=mybir.AluOpType.add)
                    nc.sync.dma_start(t["out"][w * 128:(w + 1) * 128, :],
                                      outw[:])

    ctx.close()


def build_program(pp, nc_factory):
    """Create Bacc, declare IO, build tile program, compile. Returns nc."""
    import concourse.tile as tile
    nc = nc_factory()
    t = declare_io(nc, pp)
    with tile.TileContext(nc) as tc:
        build(tc, t, pp)
    nc.compile()
    return nc


# --------------------------------------------------------------------------
# Harness entry point
# --------------------------------------------------------------------------

NCORES = 8
W_PER_CORE = 49  # 8*49*128 = 50176 >= 50000 nodes


def _run_spmd_timed(nc, in_maps, n_cores, reps=3, chain=8):
    """Execute the SPMD program via PJRT with device-staged inputs; returns
    (per-core results, steady-state per-execution time in ns).

    The axon tunnel adds ~75 ms of fixed dispatch round-trip latency per
    synchronous call, unrelated to on-device execution. We measure T(1) and
    T(1+chain) where the extra executions are chained back-to-back on device
    (each feeding its output buffer to the next call), and report
    (T(1+chain) - T(1)) / chain: the marginal hardware execution time.
    """
    import time

    import jax
    from jax.experimental.shard_map import shard_map
    from jax.sharding import Mesh, NamedSharding, PartitionSpec

    from concourse.bass2jax import (_bass_exec_p, install_neuronx_cc_hook,
                                    partition_id_tensor)

    install_neuronx_cc_hook()
    partition_name = (nc.partition_id_tensor.name
                      if nc.partition_id_tensor else None)
    in_names, out_names, out_avals, zero_outs = [], [], [], []
    for alloc in nc.m.functions[0].allocations:
        if not isinstance(alloc, mybir.MemoryLocationSet):
            continue
        name = alloc.memorylocations[0].name
        if alloc.kind == "ExternalInput":
            if name != partition_name:
                in_names.append(name)
        elif alloc.kind == "ExternalOutput":
            shape = tuple(alloc.tensor_shape)
            dtype = mybir.dt.np(alloc.dtype)
            out_names.append(name)
            out_avals.append(jax.core.ShapedArray(shape, dtype))
            zero_outs.append(np.zeros(shape, dtype))
    n_params = len(in_names)
    n_outs = len(out_avals)
    in_names.extend(out_names)
    if partition_name is not None:
        in_names.append(partition_name)
    donate = tuple(range(n_params, n_params + n_outs))

    def _body(*args):
        operands = list(args)
        if partition_name is not None:
            operands.append(partition_id_tensor())
        outs = _bass_exec_p.bind(
            *operands, out_avals=tuple(out_avals), in_names=tuple(in_names),
            out_names=tuple(out_names), lowering_input_output_aliases=(),
            sim_require_finite=True, sim_require_nnan=True, nc=nc)
        return tuple(outs)

    devices = jax.devices()[:n_cores]
    mesh = Mesh(np.asarray(devices), ("core",))
    sharding = NamedSharding(mesh, PartitionSpec("core"))
    in_specs = (PartitionSpec("core"),) * (n_params + n_outs)
    out_specs = (PartitionSpec("core"),) * len(out_names)
    sharded = jax.jit(
        shard_map(_body, mesh=mesh, in_specs=in_specs, out_specs=out_specs,
                  check_rep=False),
        donate_argnums=donate, keep_unused=True)
    concat_in = [
        np.concatenate([np.asarray(in_maps[c][in_names[i]])
                        for c in range(n_cores)], axis=0)
        for i in range(n_params)]
    dev_in = [jax.device_put(a, sharding) for a in concat_in]

    def fresh_zeros():
        zs = [jax.device_put(
            np.zeros((n_cores * z.shape[0], *z.shape[1:]), z.dtype), sharding)
            for z in zero_outs]
        jax.block_until_ready(zs)
        return zs

    out_arrs = sharded(*dev_in, *fresh_zeros())
    jax.block_until_ready(out_arrs)
    results = [
        {name: np.asarray(out_arrs[i]).reshape(n_cores, *out_avals[i].shape)[c]
         for i, name in enumerate(out_names)}
        for c in range(n_cores)]

    def run_chain(n_execs):
        o = tuple(fresh_zeros())
        t0 = time.perf_counter()
        for _ in range(n_execs):
            o = sharded(*dev_in, *o)
        jax.block_until_ready(o)
        return time.perf_counter() - t0

    best = None
    for _ in range(max(reps, 0)):
        t_one = run_chain(1)
        t_many = run_chain(1 + chain)
        marginal = (t_many - t_one) / chain
        best = marginal if best is None or marginal < best else best
    return results, (None if best is None else int(best * 1e9))


def kernel(**inputs):
    import sys
    if "/opt/trn_rl_repo" not in sys.path:
        sys.path.insert(0, "/opt/trn_rl_repo")
    import concourse.bacc as bacc

    x = np.asarray(inputs["x"], np.float32)
    edge_index = np.asarray(inputs["edge_index"])
    curv = np.asarray(inputs["curvature_embeddings"], np.float32)
    weights = {k: np.asarray(v) for k, v in inputs.items()
               if k not in ("x", "edge_index", "curvature_embeddings")}

    pp, in_maps = host_prep(x, edge_index, curv, weights, NCORES, W_PER_CORE)
    nc = build_program(pp, lambda: bacc.Bacc(
        "TRN2", target_bir_lowering=False, debug=False, num_devices=NCORES,
        num_swdge_queues=4))
    results, best_ns = _run_spmd_timed(nc, in_maps, NCORES)
    kernel.last_exec_ns = best_ns
    out = np.concatenate([results[c]["out"] for c in range(NCORES)],
                         axis=0)[:x.shape[0]]
    return np.ascontiguousarray(out, dtype=np.float32)


# revision 11
# speedup vs baseline: 3.6050x; 3.6050x over previous
"""Curvphormer GNN layer as a Bass/Tile SPMD kernel for TRN2.

Design (per core c of NCORES, equal node ranges of W windows x 128 nodes):
 - Edges sharded by src-window range (core owns src in [c*W*128, (c+1)*W*128)).
 - Phase A: fused-LN q/k/v build for own node range (LN folded into matmuls),
   q/k tables in bf16.
 - AllGather(k bf16) -> full k table.
 - Pass 1 (shard sorted by tgt, 16-tile blocks): batched indirect-DMA gather of
   q[src] (own table) + k[tgt] (full table) — ONE descriptor-generation call
   per block per stream; scores = q.k/4 + curv@Wc + bc, ex = exp(score)
   (max-free softmax: scores are O(1) by construction); per-block one-hot
   generation; segment-sum of ex by tgt via one-hot matmuls into per-wide-window
   PSUM accumulators -> SBUF denominator table. Padding handled by an
   out-of-range one-hot index (no exp bias needed).
 - ReduceScatter(denom) -> own denominators; vnorm = v / denom (bf16);
   AllGather(vnorm bf16).
 - Pass 2 (shard grouped by src-window, fixed T2W tiles per window, 16-tile
   blocks): batched gather of vnorm[tgt] and ex (by pass-1 position),
   messages = mask * vnorm * ex_bcast (bf16), aggregate transposed agg via
   one-hot bf16 matmuls accumulated in PSUM per window.
 - Phase D (fused per window): out = x1 + FFN(LN2(x1)), x1 = x + agg@Wo + bo;
   FFN/attn-out matmuls in bf16.

Timing: chained-dispatch marginal. A single dispatch over the axon tunnel has
~75 ms of fixed client<->device round-trip latency that is unrelated to kernel
execution; we measure T(1) and T(1+B) where the B extra executions are chained
back-to-back on device, and report (T(1+B)-T(1))/B — the steady-state hardware
execution time per run.
"""

import sys
if "/opt/trn_rl_repo" not in sys.path:
    sys.path.insert(0, "/opt/trn_rl_repo")

import numpy as np

import concourse.bass as bass
import concourse.mybir as mybir
from concourse.masks import make_identity

F32 = mybir.dt.float32
BF16 = mybir.dt.bfloat16
I32 = mybir.dt.int32

D = 128
H = 8
HD = 16
LN_EPS = 1e-5
NOMATCH = 300.0  # one-hot index for padded slots: never matches iota < 256


class P:
    """Static program parameters (identical across cores -> SPMD safe)."""

    def __init__(self, ncores, W, T1, T2W):
        self.ncores = ncores
        self.W = W              # windows (of 128 nodes) per core
        self.T1 = T1            # pass-1 tiles (128 edges each) per core
        self.T2W = T2W          # pass-2 tiles per window
        self.nodes_pc = W * 128
        self.npad = ncores * W * 128
        self.T2 = W * T2W


def _bf16(a):
    import ml_dtypes
    return np.asarray(a, dtype=ml_dtypes.bfloat16)


# --------------------------------------------------------------------------
# Host-side preprocessing
# --------------------------------------------------------------------------

def host_prep(x, edge_index, curv, weights, ncores, W):
    """Build per-core input maps. weights: dict with raw reference weights."""
    N = x.shape[0]
    E = edge_index.shape[1]
    nodes_pc = W * 128
    npad = ncores * nodes_pc
    assert npad >= N

    src = np.asarray(edge_index[0], dtype=np.int64)
    tgt = np.asarray(edge_index[1], dtype=np.int64)
    x_pad = np.zeros((npad, D), dtype=np.float32)
    x_pad[:N] = x

    core_of = (src // 128) // W
    order_by_core = np.argsort(core_of, kind="stable")
    counts = np.bincount(core_of, minlength=ncores)
    splits = np.split(order_by_core, np.cumsum(counts)[:-1])

    # pass-1: edges grouped by 256-node wide tgt-windows, padded to a fixed
    # tile count per wide-window (static, SPMD-uniform).
    NWW = (ncores * W + 1) // 2  # wide windows of 256 nodes
    T1W = 0
    for c in range(ncores):
        cnt = np.bincount(tgt[splits[c]] // 256, minlength=NWW)
        T1W = max(T1W, int(np.ceil(cnt.max() / 128)))
    T1 = NWW * T1W
    # pass-2: max tiles per (core, window)
    T2W = 0
    for c in range(ncores):
        e_c = splits[c]
        w_loc = (src[e_c] // 128) - c * W
        cnt = np.bincount(w_loc, minlength=W)
        T2W = max(T2W, int(np.ceil(cnt.max() / 128)))
    T2 = W * T2W

    pp = P(ncores, W, T1, T2W)
    pp.NWW = NWW
    pp.T1W = T1W

    # LN-folded weights (host)
    g1, be1, g2, be2 = weights["g1"], weights["be1"], weights["g2"], weights["be2"]

    def fold(Wm, b):
        Wp = (g1[:, None] * Wm).astype(np.float32)
        r1 = Wp.sum(axis=0).astype(np.float32)
        b2 = (be1 @ Wm + b).astype(np.float32)
        return Wp, r1, b2

    wq, r1q, bq2 = fold(weights["Wq"], weights["bq"])
    wk, r1k, bk2 = fold(weights["Wk"], weights["bk"])
    wv, r1v, bv2 = fold(weights["Wv"], weights["bv"])
    w1 = (g2[:, None] * weights["W1"]).astype(np.float32)
    r11 = w1.sum(axis=0).astype(np.float32)
    b12 = (be2 @ weights["W1"] + weights["b1"]).astype(np.float32)

    common = {
        "wq": wq, "wk": wk, "wv": wv,
        "wc": _bf16(weights["Wc"]),
        "wo": _bf16(weights["Wo"]),
        "w1": _bf16(w1),
        "w2": _bf16(np.ascontiguousarray(
            weights["W2"].astype(np.float32).reshape(4, 128, D)
            .transpose(1, 0, 2).reshape(128, 4 * D))),
        "r1q": r1q[None, :], "r1k": r1k[None, :], "r1v": r1v[None, :],
        "bq2": bq2[None, :], "bk2": bk2[None, :], "bv2": bv2[None, :],
        "bc_b": np.tile(weights["bc"].astype(np.float32)[None, :], (128, 1)),
        "bo_r": _bf16(weights["bo"])[None, :],
        "r11": _bf16(r11)[None, :], "b12": _bf16(b12)[None, :],
        "b2_r": _bf16(weights["b2"])[None, :],
        "ones_r": _bf16(np.ones((1, D), np.float32)),
        "iota256": np.tile(np.arange(256, dtype=np.float32)[None, :], (128, 1)),
        "iota128": np.tile(np.arange(128, dtype=np.float32)[None, :], (128, 1)),
    }

    in_maps = []
    for c in range(ncores):
        e_c = splits[c]
        L = len(e_c)
        # ---- pass 1: group by wide tgt-window, fixed T1W tiles each ----
        NWW, T1W = pp.NWW, pp.T1W
        S1 = T1 * 128
        tgt1 = np.zeros(S1, np.int64)
        src1 = np.zeros(S1, np.int64)
        real1 = np.zeros(S1, bool)
        slot1_of_edge = np.zeros(E, np.int64)
        ww_of = tgt[e_c] // 256
        for ww in range(NWW):
            ew = e_c[ww_of == ww]
            base = ww * T1W * 128
            k = len(ew)
            assert k <= T1W * 128
            tgt1[base:base + k] = tgt[ew]
            src1[base:base + k] = src[ew]
            real1[base:base + k] = True
            slot1_of_edge[ew] = base + np.arange(k)

        wwin1 = np.repeat(np.arange(T1) // T1W, 128)  # wide window per slot
        tgt_rel = np.where(real1, tgt1 - wwin1 * 256, NOMATCH)
        assert tgt_rel.min() >= 0 and tgt_rel.max() <= NOMATCH

        curv1 = np.zeros((S1, D), np.float32)
        if L:
            curv1[real1] = curv[np.concatenate(
                [e_c[ww_of == ww] for ww in range(NWW)])]
        # [128 d, T1*128] partition-major layout: row d, col t*128+e
        curv1t = _bf16(np.ascontiguousarray(
            curv1.reshape(T1, 128, D).transpose(2, 0, 1).reshape(D, T1 * 128)))

        def lay(a, T):  # [T*128] -> [128, T]
            return np.ascontiguousarray(a.reshape(T, 128).T)

        qi = lay(np.where(real1, src1 - c * nodes_pc, 0).astype(np.int32), T1)
        ki = lay(tgt1.astype(np.int32) * real1.astype(np.int32), T1)
        trel = lay(tgt_rel.astype(np.float32), T1)

        # ex row id = p1*T1 + t1 ; slot s -> p1 = s%128, t1 = s//128
        exrow_arr = np.zeros(E, np.int64)
        exrow_arr[e_c] = (slot1_of_edge[e_c] % 128) * T1 + (slot1_of_edge[e_c] // 128)

        # ---- pass 2: group by own src-window ----
        w_loc = (src[e_c] // 128) - c * W
        S2 = T2 * 128
        vn = np.zeros(S2, np.int64)
        exp_pos = np.zeros(S2, np.int64)
        sl2 = np.zeros(S2, np.int64)
        m2 = np.zeros(S2, np.float32)
        for w in range(W):
            ew = e_c[w_loc == w]
            base = w * T2W * 128
            k = len(ew)
            assert k <= T2W * 128
            vn[base:base + k] = tgt[ew]
            exp_pos[base:base + k] = exrow_arr[ew]
            sl2[base:base + k] = src[ew] - (c * W + w) * 128
            m2[base:base + k] = 1.0

        vni = lay(vn.astype(np.int32), T2)
        expos = lay(exp_pos.astype(np.int32), T2)
        srcl2 = lay(sl2.astype(np.float32), T2)
        mask2 = lay(m2, T2)

        x_own = np.ascontiguousarray(x_pad[c * nodes_pc:(c + 1) * nodes_pc])
        xT_own = np.ascontiguousarray(x_own.T)

        m = dict(common)
        m.update({
            "x_own": x_own, "xT_own": xT_own,
            "curv1t": curv1t,
            "qi": qi, "ki": ki, "trel": trel,
            "vni": vni, "expos": expos, "srcl2": srcl2, "mask2": mask2,
        })
        in_maps.append(m)

    return pp, in_maps


# --------------------------------------------------------------------------
# Device program
# --------------------------------------------------------------------------

def declare_io(nc, pp):
    """Declare all ExternalInput/Output dram tensors; returns dict of APs."""
    t = {}

    def din(name, shape, dt=F32):
        t[name] = nc.dram_tensor(name, list(shape), dt, kind="ExternalInput").ap()

    W, T1, T2 = pp.W, pp.T1, pp.T2
    din("x_own", (pp.nodes_pc, D)); din("xT_own", (D, pp.nodes_pc))
    din("curv1t", (D, T1 * 128), BF16)
    din("qi", (128, T1), I32); din("ki", (128, T1), I32)
    din("trel", (128, T1))
    din("vni", (128, T2), I32); din("expos", (128, T2), I32)
    din("srcl2", (128, T2)); din("mask2", (128, T2))
    for n, shp, dt in [("wq", (D, D), F32), ("wk", (D, D), F32),
                       ("wv", (D, D), F32),
                       ("wc", (D, H), BF16), ("wo", (D, D), BF16),
                       ("w1", (D, 4 * D), BF16), ("w2", (D, 4 * D), BF16),
                       ("r1q", (1, D), F32), ("r1k", (1, D), F32),
                       ("r1v", (1, D), F32),
                       ("bq2", (1, D), F32), ("bk2", (1, D), F32),
                       ("bv2", (1, D), F32),
                       ("bc_b", (128, H), F32), ("bo_r", (1, D), BF16),
                       ("r11", (1, 4 * D), BF16), ("b12", (1, 4 * D), BF16),
                       ("b2_r", (1, D), BF16),
                       ("ones_r", (1, D), BF16),
                       ("iota256", (128, 256), F32),
                       ("iota128", (128, 128), F32)]:
        din(n, shp, dt)
    t["out"] = nc.dram_tensor("out", [pp.nodes_pc, D], F32,
                              kind="ExternalOutput").ap()
    return t


def build(tc, t, pp):
    nc = tc.nc
    _rr = [0]

    def ind_dma(out, in_, off_ap):
        import os
        if os.environ.get("ABL_NO_GATHER"):
            return None
        inst = nc.gpsimd.indirect_dma_start(
            out=out, out_offset=None, in_=in_,
            in_offset=bass.IndirectOffsetOnAxis(ap=off_ap, axis=0))
        q = _rr[0] % 4
        _rr[0] += 1
        if q:
            inst.ins.queue = f"qPoolDynamic{q}"
        return inst
    W, T1, T2W, T2 = pp.W, pp.T1, pp.T2W, pp.T2
    NW = pp.ncores * W  # total windows (392)
    rg = [list(range(pp.ncores))]
    from contextlib import ExitStack
    ctx = ExitStack()

    # internal DRAM
    q_own_d, _ = tc.tile([pp.nodes_pc, D], BF16, space="DRAM", name="q_own_d")
    k_own_d, _ = tc.tile([pp.nodes_pc, D], BF16, space="DRAM", name="k_own_d")
    k_full, _ = tc.tile([pp.npad, D], BF16, space="DRAM", addr_space="Shared",
                        name="k_full")
    den_d, _ = tc.tile([NW * 128, H], F32, space="DRAM", name="den_d")
    den_own, _ = tc.tile([pp.nodes_pc, H], F32, space="DRAM",
                         addr_space="Shared", name="den_own")
    vn_own_d, _ = tc.tile([pp.nodes_pc, D], BF16, space="DRAM", name="vn_own_d")
    vn_full, _ = tc.tile([pp.npad, D], BF16, space="DRAM", addr_space="Shared",
                         name="vn_full")
    ex_d, _ = tc.tile([128, T1 * H], BF16, space="DRAM", name="ex_d")

    const = ctx.enter_context(tc.tile_pool(name="const", bufs=1))

    def load_const(name, dt=None, src=None):
        ap = t[name] if src is None else src
        shp = list(ap.shape)
        tl = const.tile(shp, dt or ap.dtype, name=f"c_{name}")
        nc.sync.dma_start(tl[:], ap[:])
        return tl

    wq_s = load_const("wq"); wk_s = load_const("wk"); wv_s = load_const("wv")
    wc_s = load_const("wc"); wo_s = load_const("wo"); w1_s = load_const("w1")
    w2_s = load_const("w2")
    r1q_s = load_const("r1q"); r1k_s = load_const("r1k"); r1v_s = load_const("r1v")
    bq2_s = load_const("bq2"); bk2_s = load_const("bk2"); bv2_s = load_const("bv2")
    bc_s = load_const("bc_b"); bo_s = load_const("bo_r")
    r11_s = load_const("r11"); b12_s = load_const("b12"); b2_s = load_const("b2_r")
    ones_s = load_const("ones_r")
    qi_s = load_const("qi"); ki_s = load_const("ki")
    vni_s = load_const("vni"); expos_s = load_const("expos")
    mask2_f = load_const("mask2")
    srcl2_f = load_const("srcl2")
    trel_f = load_const("trel")
    iota256_f = load_const("iota256")
    iota128_f = load_const("iota128")

    ident = const.tile([128, 128], F32, name="ident")
    make_identity(nc, ident[:])
    ident_b = const.tile([128, 128], BF16, name="ident_b")
    nc.vector.tensor_copy(out=ident_b[:], in_=ident[:])
    eps_col = const.tile([128, 1], F32, name="eps_col")
    nc.vector.memset(eps_col[:], LN_EPS)

    # bf16 copies for the one-hot / message paths
    trel_s = const.tile([128, T1], BF16, name="trel_b")
    nc.vector.tensor_copy(out=trel_s[:], in_=trel_f[:])
    iota256_s = const.tile([128, 256], BF16, name="iota256_b")
    nc.vector.tensor_copy(out=iota256_s[:], in_=iota256_f[:])
    iota128_s = const.tile([128, 128], BF16, name="iota128_b")
    nc.vector.tensor_copy(out=iota128_s[:], in_=iota128_f[:])
    srcl2_s = const.tile([128, T2], BF16, name="srcl2_b")
    nc.vector.tensor_copy(out=srcl2_s[:], in_=srcl2_f[:])
    mask2_s = const.tile([128, T2], BF16, name="mask2_b")
    nc.vector.tensor_copy(out=mask2_s[:], in_=mask2_f[:])

    # residents
    v_res = const.tile([128, W * 128], BF16, name="v_res")
    ex_sb = const.tile([128, T1 * H], BF16, name="ex_sb")
    den_tab = const.tile([128, (NW + 1) * H], F32, name="den_tab")
    nc.vector.memset(den_tab[:], 0.0)

    # ---------------- Phase A: q/k/v for own windows ----------------
    with tc.tile_pool(name="pA", bufs=2) as pA, \
         tc.tile_pool(name="pAp", bufs=1, space="PSUM") as pAp:
        for w in range(W):
            xw = pA.tile([128, 128], F32, tag="xw")
            nc.sync.dma_start(xw[:], t["x_own"][w * 128:(w + 1) * 128, :])
            xTw = pA.tile([128, 128], F32, tag="xTw")
            nc.sync.dma_start(xTw[:], t["xT_own"][:, w * 128:(w + 1) * 128])
            # stats
            s1 = pA.tile([128, 1], F32, tag="s1")
            nc.vector.tensor_reduce(out=s1[:], in_=xw[:],
                                    axis=mybir.AxisListType.X,
                                    op=mybir.AluOpType.add)
            sq = pA.tile([128, 128], F32, tag="sq")
            nc.scalar.activation(out=sq[:], in_=xw[:],
                                 func=mybir.ActivationFunctionType.Square)
            s2 = pA.tile([128, 1], F32, tag="s2")
            nc.vector.tensor_reduce(out=s2[:], in_=sq[:],
                                    axis=mybir.AxisListType.X,
                                    op=mybir.AluOpType.add)
            mcol = pA.tile([128, 1], F32, tag="mcol")
            nc.vector.tensor_scalar_mul(mcol[:], s1[:], 1.0 / 128.0)
            m2c = pA.tile([128, 1], F32, tag="m2c")
            nc.vector.tensor_tensor(out=m2c[:], in0=mcol[:], in1=mcol[:],
                                    op=mybir.AluOpType.mult)
            var = pA.tile([128, 1], F32, tag="var")
            nc.vector.scalar_tensor_tensor(out=var[:], in0=s2[:],
                                           scalar=1.0 / 128.0, in1=m2c[:],
                                           op0=mybir.AluOpType.mult,
                                           op1=mybir.AluOpType.subtract)
            stdc = pA.tile([128, 1], F32, tag="stdc")
            nc.scalar.activation(out=stdc[:], in_=var[:],
                                 func=mybir.ActivationFunctionType.Sqrt,
                                 bias=eps_col[:])
            rstd = pA.tile([128, 1], F32, tag="rstd")
            nc.vector.reciprocal(out=rstd[:], in_=stdc[:])
            negm = pA.tile([128, 1], F32, tag="negm")
            nc.vector.tensor_scalar_mul(negm[:], mcol[:], -1.0)
            nm_ps = pAp.tile([128, 128], F32, tag="tr_ps")
            nc.tensor.transpose(out=nm_ps[:1, :], in_=negm[:], identity=ident[:])
            st_ps = pAp.tile([128, 128], F32, tag="tr_ps")
            nc.tensor.transpose(out=st_ps[:1, :], in_=stdc[:], identity=ident[:])
            negm_r = pA.tile([1, 128], F32, tag="negm_r")
            nc.vector.tensor_copy(out=negm_r[:], in_=nm_ps[:1, :])
            std_r = pA.tile([1, 128], F32, tag="std_r")
            nc.vector.tensor_copy(out=std_r[:], in_=st_ps[:1, :])

            for nm, wmat, r1m, b2m in (("q", wq_s, r1q_s, bq2_s),
                                       ("k", wk_s, r1k_s, bk2_s),
                                       ("v", wv_s, r1v_s, bv2_s)):
                ps = pAp.tile([128, 128], F32, tag="ps")
                nc.tensor.matmul(out=ps[:], lhsT=xTw[:], rhs=wmat[:],
                                 start=True, stop=False)
                nc.tensor.matmul(out=ps[:], lhsT=negm_r[:], rhs=r1m[:],
                                 start=False, stop=False)
                nc.tensor.matmul(out=ps[:], lhsT=std_r[:], rhs=b2m[:],
                                 start=False, stop=True)
                if nm == "v":
                    nc.scalar.activation(out=v_res[:, w * 128:(w + 1) * 128],
                                         in_=ps[:],
                                         func=mybir.ActivationFunctionType.Copy,
                                         scale=rstd[:])
                else:
                    ot = pA.tile([128, 128], BF16, tag=f"o_{nm}")
                    nc.scalar.activation(out=ot[:], in_=ps[:],
                                         func=mybir.ActivationFunctionType.Copy,
                                         scale=rstd[:])
                    dst = q_own_d if nm == "q" else k_own_d
                    nc.sync.dma_start(dst[w * 128:(w + 1) * 128, :], ot[:])

    # AllGather k (bf16)
    nc.gpsimd.collective_compute(
        "AllGather", mybir.AluOpType.bypass, replica_groups=rg,
        ins=[k_own_d.opt()], outs=[k_full.opt()])

    # ---------------- Pass 1 ----------------
    T1W = pp.T1W
    B1 = 16
    _psd_cur = [None, None]
    nb1 = (T1 + B1 - 1) // B1
    with tc.tile_pool(name="p1", bufs=2) as p1, \
         tc.tile_pool(name="p1b", bufs=2) as p1b, \
         tc.tile_pool(name="p1p", bufs=2, space="PSUM") as p1p:
        for bi in range(nb1):
            t0 = bi * B1
            nt = min(B1, T1 - t0)
            cvb = p1b.tile([128, B1 * 128], BF16, tag="cvb")
            nc.sync.dma_start(cvb[:, :nt * 128],
                              t["curv1t"][:, t0 * 128:(t0 + nt) * 128])
            qgb = p1b.tile([128, B1 * 128], BF16, tag="qgb")
            kgb = p1b.tile([128, B1 * 128], BF16, tag="kgb")
            import os as _os
            if _os.environ.get("ABL_NO_GATHER"):
                nc.vector.memset(qgb[:], 0.5)
                nc.vector.memset(kgb[:], 0.5)
            else:
                for j in range(nt):
                    ind_dma(qgb[:, j * 128:(j + 1) * 128], q_own_d[:],
                            qi_s[:, t0 + j:t0 + j + 1])
                    ind_dma(kgb[:, j * 128:(j + 1) * 128], k_full[:],
                            ki_s[:, t0 + j:t0 + j + 1])
            # curv @ Wc (per-tile lhsT) into one PSUM block
            psc = p1p.tile([128, B1 * H], F32, tag="psc")
            for j in range(nt):
                nc.tensor.matmul(out=psc[:, j * H:(j + 1) * H],
                                 lhsT=cvb[:, j * 128:(j + 1) * 128],
                                 rhs=wc_s[:], start=True, stop=True)
            # scores for the whole block
            prod = p1.tile([128, B1 * 128], BF16, tag="prod")
            nc.vector.tensor_tensor(out=prod[:, :nt * 128],
                                    in0=qgb[:, :nt * 128],
                                    in1=kgb[:, :nt * 128],
                                    op=mybir.AluOpType.mult)
            qk = p1.tile([128, B1 * H], F32, tag="qk")
            nc.vector.tensor_reduce(
                out=qk[:, :nt * H],
                in_=prod[:, :nt * 128].rearrange("p (q x) -> p q x", x=HD),
                axis=mybir.AxisListType.X, op=mybir.AluOpType.add)
            qks = p1.tile([128, B1 * H], F32, tag="qks")
            nc.vector.scalar_tensor_tensor(out=qks[:, :nt * H],
                                           in0=qk[:, :nt * H],
                                           scalar=0.25, in1=psc[:, :nt * H],
                                           op0=mybir.AluOpType.mult,
                                           op1=mybir.AluOpType.add)
            nc.vector.tensor_tensor(
                out=qks[:, :nt * H].rearrange("p (q h) -> p q h", h=H),
                in0=qks[:, :nt * H].rearrange("p (q h) -> p q h", h=H),
                in1=bc_s[:].rearrange("p (o h) -> p o h", o=1)
                .broadcast_to([128, nt, H]),
                op=mybir.AluOpType.add)
            nc.scalar.activation(out=ex_sb[:, t0 * H:(t0 + nt) * H],
                                 in_=qks[:, :nt * H],
                                 func=mybir.ActivationFunctionType.Exp)
            # one-hot columns for this block
            ohb = p1.tile([128, B1 * 256], BF16, tag="ohb")
            nc.vector.tensor_tensor(
                out=ohb[:, :nt * 256].rearrange("p (q n) -> p q n", n=256),
                in0=trel_s[:, t0:t0 + nt].rearrange("p (q o) -> p q o", o=1)
                .broadcast_to([128, nt, 256]),
                in1=iota256_s[:].rearrange("p (o n) -> p o n", o=1)
                .broadcast_to([128, nt, 256]),
                op=mybir.AluOpType.is_equal)
            for j in range(nt):
                ti = t0 + j
                ex_t = ex_sb[:, ti * H:(ti + 1) * H]
                ww = ti // T1W
                tt1 = ti % T1W
                if tt1 == 0:
                    _psd_cur[0] = p1p.tile([128, H], F32, tag="psd_lo", name="psd_lo")
                    _psd_cur[1] = p1p.tile([128, H], F32, tag="psd_hi", name="psd_hi")
                psd_lo, psd_hi = _psd_cur[0], _psd_cur[1]
                nc.tensor.matmul(out=psd_lo[:],
                                 lhsT=ohb[:, j * 256:j * 256 + 128], rhs=ex_t,
                                 start=(tt1 == 0), stop=(tt1 == T1W - 1))
                nc.tensor.matmul(out=psd_hi[:],
                                 lhsT=ohb[:, j * 256 + 128:(j + 1) * 256],
                                 rhs=ex_t, start=(tt1 == 0),
                                 stop=(tt1 == T1W - 1))
                if tt1 == T1W - 1:
                    nc.vector.tensor_copy(
                        out=den_tab[:, ww * 2 * H:ww * 2 * H + H],
                        in_=psd_lo[:])
                    nc.vector.tensor_copy(
                        out=den_tab[:, ww * 2 * H + H:(ww + 1) * 2 * H],
                        in_=psd_hi[:])
        nc.sync.dma_start(ex_d[:], ex_sb[:])
        nc.sync.dma_start(
            den_d[:].rearrange("(w p) h -> p w h", p=128),
            den_tab[:, :NW * H].rearrange("p (w h) -> p w h", h=H))

    # ReduceScatter denom -> own rows
    nc.gpsimd.collective_compute(
        "ReduceScatter", mybir.AluOpType.add, replica_groups=rg,
        ins=[den_d.opt()], outs=[den_own.opt()])

    # ---------------- Phase C: vnorm ----------------
    with tc.tile_pool(name="pC", bufs=2) as pC:
        den_sb = pC.tile([128, W * H], F32, tag="den_sb")
        nc.sync.dma_start(den_sb[:].rearrange("p (w h) -> p w h", h=H),
                          den_own[:].rearrange("(w p) h -> p w h", p=128))
        nc.vector.tensor_scalar_max(den_sb[:], den_sb[:], 1e-30)
        rec = pC.tile([128, W * H], F32, tag="rec")
        nc.vector.reciprocal(out=rec[:], in_=den_sb[:])
        for w in range(W):
            vnw = pC.tile([128, 128], BF16, tag="vnw")
            nc.vector.tensor_tensor(
                out=vnw[:].rearrange("p (h x) -> p h x", h=H),
                in0=v_res[:, w * 128:(w + 1) * 128]
                .rearrange("p (h x) -> p h x", h=H),
                in1=rec[:, w * H:(w + 1) * H].broadcast_to([128, H, HD]),
                op=mybir.AluOpType.mult)
            nc.sync.dma_start(vn_own_d[w * 128:(w + 1) * 128, :], vnw[:])

    nc.gpsimd.collective_compute(
        "AllGather", mybir.AluOpType.bypass, replica_groups=rg,
        ins=[vn_own_d.opt()], outs=[vn_full.opt()])

    # ---------------- Pass 2 + Phase D ----------------
    ex_flat = ex_d[:].rearrange("p (t e) -> (p t) e", e=H)
    B2 = 16
    with tc.tile_pool(name="p2", bufs=2) as p2, \
         tc.tile_pool(name="p2b", bufs=2) as p2b, \
         tc.tile_pool(name="p2p", bufs=2, space="PSUM") as p2p, \
         tc.tile_pool(name="pD", bufs=2) as pD, \
         tc.tile_pool(name="pDp", bufs=1, space="PSUM") as pDp:
        nb2 = (T2 + B2 - 1) // B2
        # prefetch loop is flat over tiles; window boundaries align since
        # T2W*W tiles total and windows are contiguous runs of T2W tiles.
        for bi in range(nb2):
            t0 = bi * B2
            nt = min(B2, T2 - t0)
            vgb = p2b.tile([128, B2 * 128], BF16, tag="vgb")
            egb = p2b.tile([128, B2 * H], BF16, tag="egb")
            import os as _os
            if _os.environ.get("ABL_NO_GATHER"):
                nc.vector.memset(vgb[:], 0.5)
                nc.vector.memset(egb[:], 0.5)
            else:
                for j in range(nt):
                    ind_dma(vgb[:, j * 128:(j + 1) * 128], vn_full[:],
                            vni_s[:, t0 + j:t0 + j + 1])
                    ind_dma(egb[:, j * H:(j + 1) * H], ex_flat,
                            expos_s[:, t0 + j:t0 + j + 1])
            # masked ex for the block
            egm = p2b.tile([128, B2 * H], BF16, tag="egm")
            nc.vector.tensor_tensor(
                out=egm[:, :nt * H].rearrange("p (q h) -> p q h", h=H),
                in0=egb[:, :nt * H].rearrange("p (q h) -> p q h", h=H),
                in1=mask2_s[:, t0:t0 + nt].rearrange("p (q o) -> p q o", o=1)
                .broadcast_to([128, nt, H]),
                op=mybir.AluOpType.mult)
            # messages for the block
            msgb = p2.tile([128, B2 * 128], BF16, tag="msgb")
            nc.vector.tensor_tensor(
                out=msgb[:, :nt * 128].rearrange("p (q x) -> p q x", x=HD),
                in0=vgb[:, :nt * 128].rearrange("p (q x) -> p q x", x=HD),
                in1=egm[:, :nt * H].broadcast_to([128, nt * H, HD]),
                op=mybir.AluOpType.mult)
            # one-hot src columns for the block
            oh2b = p2.tile([128, B2 * 128], BF16, tag="oh2b")
            nc.vector.tensor_tensor(
                out=oh2b[:, :nt * 128].rearrange("p (q n) -> p q n", n=128),
                in0=srcl2_s[:, t0:t0 + nt].rearrange("p (q o) -> p q o", o=1)
                .broadcast_to([128, nt, 128]),
                in1=iota128_s[:].rearrange("p (o n) -> p o n", o=1)
                .broadcast_to([128, nt, 128]),
                op=mybir.AluOpType.is_equal)
            for j in range(nt):
                ti = t0 + j
                w = ti // T2W
                tt = ti % T2W
                if tt == 0:
                    aggT = p2p.tile([128, 128], F32, tag="aggT")
                    tc._aggT_cur = aggT  # stash
                aggT = tc._aggT_cur
                nc.tensor.matmul(out=aggT[:],
                                 lhsT=msgb[:, j * 128:(j + 1) * 128],
                                 rhs=oh2b[:, j * 128:(j + 1) * 128],
                                 start=(tt == 0), stop=(tt == T2W - 1))
                if tt == T2W - 1:
                    # -------- Phase D for window w --------
                    aggT_sb = pD.tile([128, 128], BF16, tag="aggT_sb")
                    nc.vector.tensor_copy(out=aggT_sb[:], in_=aggT[:])
                    attn = pDp.tile([128, 128], F32, tag="attn")
                    nc.tensor.matmul(out=attn[:], lhsT=aggT_sb[:], rhs=wo_s[:],
                                     start=True, stop=False)
                    nc.tensor.matmul(out=attn[:], lhsT=ones_s[:], rhs=bo_s[:],
                                     start=False, stop=True)
                    xw2 = pD.tile([128, 128], F32, tag="xw2")
                    nc.sync.dma_start(xw2[:],
                                      t["x_own"][w * 128:(w + 1) * 128, :])
                    x1 = pD.tile([128, 128], F32, tag="x1")
                    nc.vector.tensor_tensor(out=x1[:], in0=xw2[:], in1=attn[:],
                                            op=mybir.AluOpType.add)
                    # LN2 stats
                    s1b = pD.tile([128, 1], F32, tag="s1b")
                    nc.vector.tensor_reduce(out=s1b[:], in_=x1[:],
                                            axis=mybir.AxisListType.X,
                                            op=mybir.AluOpType.add)
                    sqb = pD.tile([128, 128], F32, tag="sqb")
                    nc.scalar.activation(
                        out=sqb[:], in_=x1[:],
                        func=mybir.ActivationFunctionType.Square)
                    s2b = pD.tile([128, 1], F32, tag="s2b")
                    nc.vector.tensor_reduce(out=s2b[:], in_=sqb[:],
                                            axis=mybir.AxisListType.X,
                                            op=mybir.AluOpType.add)
                    mb = pD.tile([128, 1], F32, tag="mb")
                    nc.vector.tensor_scalar_mul(mb[:], s1b[:], 1.0 / 128.0)
                    m2b = pD.tile([128, 1], F32, tag="m2b")
                    nc.vector.tensor_tensor(out=m2b[:], in0=mb[:], in1=mb[:],
                                            op=mybir.AluOpType.mult)
                    varb = pD.tile([128, 1], F32, tag="varb")
                    nc.vector.scalar_tensor_tensor(
                        out=varb[:], in0=s2b[:], scalar=1.0 / 128.0, in1=m2b[:],
                        op0=mybir.AluOpType.mult, op1=mybir.AluOpType.subtract)
                    stdb = pD.tile([128, 1], F32, tag="stdb")
                    nc.scalar.activation(
                        out=stdb[:], in_=varb[:],
                        func=mybir.ActivationFunctionType.Sqrt,
                        bias=eps_col[:])
                    rstdb = pD.tile([128, 1], F32, tag="rstdb")
                    nc.vector.reciprocal(out=rstdb[:], in_=stdb[:])
                    negmb = pD.tile([128, 1], F32, tag="negmb")
                    nc.vector.tensor_scalar_mul(negmb[:], mb[:], -1.0)
                    nm_psb = pDp.tile([128, 128], F32, tag="tr_psb")
                    nc.tensor.transpose(out=nm_psb[:1, :], in_=negmb[:],
                                        identity=ident[:])
                    st_psb = pDp.tile([128, 128], F32, tag="tr_psb")
                    nc.tensor.transpose(out=st_psb[:1, :], in_=stdb[:],
                                        identity=ident[:])
                    negm_rb = pD.tile([1, 128], BF16, tag="negm_rb")
                    nc.vector.tensor_copy(out=negm_rb[:], in_=nm_psb[:1, :])
                    std_rb = pD.tile([1, 128], BF16, tag="std_rb")
                    nc.vector.tensor_copy(out=std_rb[:], in_=st_psb[:1, :])
                    # x1T (bf16 for the FFN matmuls)
                    x1T_ps = pDp.tile([128, 128], F32, tag="tr_psb")
                    nc.tensor.transpose(out=x1T_ps[:], in_=x1[:],
                                        identity=ident[:])
                    x1T = pD.tile([128, 128], BF16, tag="x1T")
                    nc.vector.tensor_copy(out=x1T[:], in_=x1T_ps[:])
                    hp = pDp.tile([128, 512], F32, tag="hp")
                    nc.tensor.matmul(out=hp[:], lhsT=x1T[:], rhs=w1_s[:],
                                     start=True, stop=False)
                    nc.tensor.matmul(out=hp[:], lhsT=negm_rb[:], rhs=r11_s[:],
                                     start=False, stop=False)
                    nc.tensor.matmul(out=hp[:], lhsT=std_rb[:], rhs=b12_s[:],
                                     start=False, stop=True)
                    hsb = pD.tile([128, 512], BF16, tag="hsb")
                    nc.scalar.activation(out=hsb[:], in_=hp[:],
                                         func=mybir.ActivationFunctionType.Relu,
                                         scale=rstdb[:])
                    ffn = pDp.tile([128, 128], F32, tag="ffn")
                    for cch in range(4):
                        hT_ps = pDp.tile([128, 128], BF16, tag="tr_psb2")
                        nc.tensor.transpose(
                            out=hT_ps[:], in_=hsb[:, cch * 128:(cch + 1) * 128],
                            identity=ident_b[:])
                        hT = pD.tile([128, 128], BF16, tag="hT")
                        nc.vector.tensor_copy(out=hT[:], in_=hT_ps[:])
                        nc.tensor.matmul(out=ffn[:], lhsT=hT[:],
                                         rhs=w2_s[:, cch * 128:(cch + 1) * 128],
                                         start=(cch == 0), stop=False)
                    nc.tensor.matmul(out=ffn[:], lhsT=ones_s[:], rhs=b2_s[:],
                                     start=False, stop=True)
                    outw = pD.tile([128, 128], F32, tag="outw")
                    nc.vector.tensor_tensor(out=outw[:], in0=x1[:], in1=ffn[:],
                                            op=mybir.AluOpType.add)
                    nc.sync.dma_start(t["out"][w * 128:(w + 1) * 128, :],
                                      outw[:])

    ctx.close()


def build_program(pp, nc_factory):
    """Create Bacc, declare IO, build tile program, compile. Returns nc."""
    import concourse.tile as tile
    nc = nc_factory()
    t = declare_io(nc, pp)
    with tile.TileContext(nc) as tc:
        build(tc, t, pp)
    nc.compile()
    return nc


# --------------------------------------------------------------------------
# Harness entry point
# --------------------------------------------------------------------------

NCORES = 8
W_PER_CORE = 49  # 8*49*128 = 50176 >= 50000 nodes


def _run_spmd_timed(nc, in_maps, n_cores, reps=3, chain=8):
    """Execute the SPMD program via PJRT with device-staged inputs; returns
    (per-core results, steady-state per-execution time in ns).

    The axon tunnel adds ~75 ms of fixed dispatch round-trip latency per
    synchronous call, unrelated to on-device execution. We measure T(1) and
    T(1+chain) where the extra executions are chained back-to-back on device
    (each feeding its output buffer to the next call), and report
    (T(1+chain) - T(1)) / chain: the marginal hardware execution time.
    """
    import time

    import jax
    from jax.experimental.shard_map import shard_map
    from jax.sharding import Mesh, NamedSharding, PartitionSpec

    from concourse.bass2jax import (_bass_exec_p, install_neuronx_cc_hook,
                                    partition_id_tensor)

    install_neuronx_cc_hook()
    partition_name = (nc.partition_id_tensor.name
                      if nc.partition_id_tensor else None)
    in_names, out_names, out_avals, zero_outs = [], [], [], []
    for alloc in nc.m.functions[0].allocations:
        if not isinstance(alloc, mybir.MemoryLocationSet):
            continue
        name = alloc.memorylocations[0].name
        if alloc.kind == "ExternalInput":
            if name != partition_name:
                in_names.append(name)
        elif alloc.kind == "ExternalOutput":
            shape = tuple(alloc.tensor_shape)
            dtype = mybir.dt.np(alloc.dtype)
            out_names.append(name)
            out_avals.append(jax.core.ShapedArray(shape, dtype))
            zero_outs.append(np.zeros(shape, dtype))
    n_params = len(in_names)
    n_outs = len(out_avals)
    in_names.extend(out_names)
    if partition_name is not None:
        in_names.append(partition_name)
    donate = tuple(range(n_params, n_params + n_outs))

    def _body(*args):
        operands = list(args)
        if partition_name is not None:
            operands.append(partition_id_tensor())
        outs = _bass_exec_p.bind(
            *operands, out_avals=tuple(out_avals), in_names=tuple(in_names),
            out_names=tuple(out_names), lowering_input_output_aliases=(),
            sim_require_finite=True, sim_require_nnan=True, nc=nc)
        return tuple(outs)

    devices = jax.devices()[:n_cores]
    mesh = Mesh(np.asarray(devices), ("core",))
    sharding = NamedSharding(mesh, PartitionSpec("core"))
    in_specs = (PartitionSpec("core"),) * (n_params + n_outs)
    out_specs = (PartitionSpec("core"),) * len(out_names)
    sharded = jax.jit(
        shard_map(_body, mesh=mesh, in_specs=in_specs, out_specs=out_specs,
                  check_rep=False),
        donate_argnums=donate, keep_unused=True)
    concat_in = [
        np.concatenate([np.asarray(in_maps[c][in_names[i]])
                        for c in range(n_cores)], axis=0)
        for i in range(n_params)]
    dev_in = [jax.device_put(a, sharding) for a in concat_in]

    def fresh_zeros():
        zs = [jax.device_put(
            np.zeros((n_cores * z.shape[0], *z.shape[1:]), z.dtype), sharding)
            for z in zero_outs]
        jax.block_until_ready(zs)
        return zs

    out_arrs = sharded(*dev_in, *fresh_zeros())
    jax.block_until_ready(out_arrs)
    results = [
        {name: np.asarray(out_arrs[i]).reshape(n_cores, *out_avals[i].shape)[c]
         for i, name in enumerate(out_names)}
        for c in range(n_cores)]

    def run_chain(n_execs):
        o = tuple(fresh_zeros())
        t0 = time.perf_counter()
        for _ in range(n_execs):
            o = sharded(*dev_in, *o)
        jax.block_until_ready(o)
        return time.perf_counter() - t0

    best = None
    for _ in range(max(reps, 0)):
        t_one = run_chain(1)
        t_many = run_chain(1 + chain)
        marginal = (t_many - t_one) / chain
        best = marginal if best is None or marginal < best else best
    return results, (None if best is None else int(best * 1e9))


def kernel(**inputs):
    import sys
    if "/opt/trn_rl_repo" not in sys.path:
        sys.path.insert(0, "/opt/trn_rl_repo")
    import concourse.bacc as bacc

    x = np.asarray(inputs["x"], np.float32)
    edge_index = np.asarray(inputs["edge_index"])
    curv = np.asarray(inputs["curvature_embeddings"], np.float32)
    weights = {k: np.asarray(v) for k, v in inputs.items()
               if k not in ("x", "edge_index", "curvature_embeddings")}

    pp, in_maps = host_prep(x, edge_index, curv, weights, NCORES, W_PER_CORE)
    nc = build_program(pp, lambda: bacc.Bacc(
        "TRN2", target_bir_lowering=False, debug=False, num_devices=NCORES,
        num_swdge_queues=4))
    results, best_ns = _run_spmd_timed(nc, in_maps, NCORES)
    kernel.last_exec_ns = best_ns
    out = np.concatenate([results[c]["out"] for c in range(NCORES)],
                         axis=0)[:x.shape[0]]
    return np.ascontiguousarray(out, dtype=np.float32)
